# revision 3
# baseline (speedup 1.0000x reference)
"""Walsh-Hadamard transform (4096-point, orthonormal) on trn2, 8 cores.

y[r] = (H_4096 @ x[r]) / 64  for each of 16384 rows.

Scheme: H_4096 = H_8 (x) H_2 (x) H_256 over n = i*512 + d*256 + e.
Rows are processed in groups of 16 as [128 partitions = (rr*8 + i),
free = j = (d,e) in 512]; each partition row is one contiguous 2 KiB
chunk of DRAM, which halves the DMA descriptor count vs a 1 KiB split
(per-descriptor overhead is what binds the DMA engines).

On-chip compute runs in bf16 (1 PE cycle/col vs 4 for fp32); the H_2
factor is folded into mm2's moving operand as [Hs, +-Hs]:
  loads : nc.gpsimd (SWDGE ring) fp32 -> bf16 cast during DMA
  mm1   : out1 = Xb.T @ BD  (BD = I_16 (x) H_8)     4 MMs N=128 / group
  mid   : ACT copy PSUM fp32 -> SBUF bf16 (t1)
  mm2   : out2[(rr,a),(b,c)] = sum_{d,s} t1[d,s].T @ [Hs_s, (-1)^d Hs_s]
          4 accumulating MMs N=512 / group  (Hs = H_256 / 64)
  final : DVE copy PSUM fp32 -> SBUF fp32
  stores: nc.sync (SP HWDGE ring) fp32
Both matmuls keep the *data* stationary so the matmul performs the
layout corner-turn; the final layout [(rr,a), (b,c)] is the natural
row-major output layout, so load and store are plain 2 KiB-chunk DMAs.
The 1/64 scale is folded into Hs (entries +-2^-6, exact in bf16).

Work is sharded row-wise: core c processes rows [c*2048, (c+1)*2048).
"""

import numpy as np

N_ROWS = 16384
DIM = 4096
N_CORES = 8
R_PER_CORE = N_ROWS // N_CORES  # 2048

I = 8    # first-stage Hadamard size (contraction of mm1)
RR = 128 // I  # 16 rows per group
J = DIM // I   # 512 = (d in 2) x (e in 256)
G = 8    # groups per DMA chunk -> 128 rows = 2 MiB (fp32) per direction
NQ = J // 128  # 4 j-chunks per group

_PROG_CACHE = {}


def _hadamard(n: int) -> np.ndarray:
    H = np.array([[1.0]], dtype=np.float64)
    while H.shape[0] < n:
        H = np.block([[H, H], [H, -H]])
    return H


def _build_program():
    import ml_dtypes
    import concourse.mybir as mybir
    from concourse import bacc
    from concourse.tile import TileContext

    f32 = mybir.dt.float32
    bf16 = mybir.dt.bfloat16
    np_bf16 = ml_dtypes.bfloat16
    nc = bacc.Bacc("TRN2")

    x = nc.declare_dram_parameter("x", [R_PER_CORE, DIM], f32, isOutput=False)
    y = nc.declare_dram_parameter("y", [R_PER_CORE, DIM], f32, isOutput=True)

    BD = np.kron(np.eye(RR), _hadamard(I)).astype(np_bf16)  # [(rr,i),(rr,a)]
    Hs = (_hadamard(256) / 64.0).astype(np.float32)  # [e, c]
    # rhs of mm2 for (d, s): [eh=128, (b,c)=512] = [Hs_s, (-1)^d Hs_s]
    Hs3 = np.empty((128, 2, 2, 512), dtype=np.float32)
    for dd in range(2):
        for s in range(2):
            blk = Hs[s * 128 : (s + 1) * 128, :]
            Hs3[:, dd, s, :256] = blk
            Hs3[:, dd, s, 256:] = blk if dd == 0 else -blk
    Hs3 = Hs3.astype(np_bf16)
    bd_d = nc.inline_tensor(BD, "bd_const")
    hs_d = nc.inline_tensor(Hs3, "hs_const")

    n_chunks = R_PER_CORE // (RR * G)  # 16

    xv = x[:].rearrange("(cb g rr) (i j) -> cb (rr i) g j", g=G, rr=RR, i=I, j=J)
    yv = y[:].rearrange(
        "(cb g rr) (a w) -> cb (rr a) g w", g=G, rr=RR, a=I, w=J
    )

    with TileContext(nc) as tc:
        with (
            tc.tile_pool(name="consts", bufs=1) as cpool,
            tc.tile_pool(name="xb", bufs=3) as xbpool,
            tc.tile_pool(name="t1", bufs=6) as t1pool,
            tc.tile_pool(name="outp", bufs=3) as outpool,
            tc.tile_pool(name="ps1", bufs=3, space="PSUM") as ps1pool,
            tc.tile_pool(name="ps2", bufs=3, space="PSUM") as ps2pool,
        ):
            bd_sb = cpool.tile([128, 128], bf16)
            hs_sb = cpool.tile([128, 2, 2, J], bf16)
            nc.sync.dma_start(out=bd_sb[:], in_=bd_d[:])
            nc.sync.dma_start(out=hs_sb[:], in_=hs_d[:])

            for cb in range(n_chunks):
                xb = xbpool.tile([128, G, J], bf16)
                for h in range(2):
                    nc.gpsimd.dma_start(
                        out=xb[:, h * (G // 2) : (h + 1) * (G // 2)],
                        in_=xv[cb][:, h * (G // 2) : (h + 1) * (G // 2)],
                    )
                out_tile = outpool.tile([128, G, J], f32)
                for g in range(G):
                    ps1 = ps1pool.tile([128, NQ, 128], f32)
                    for q in range(NQ):
                        nc.tensor.matmul(
                            ps1[:, q],
                            xb[:, g, q * 128 : (q + 1) * 128],
                            bd_sb[:],
                            start=True,
                            stop=True,
                        )
                    t1 = t1pool.tile([128, NQ, 128], bf16)
                    nc.scalar.copy(
                        t1[:].rearrange("p q k -> p (q k)"),
                        ps1[:].rearrange("p q k -> p (q k)"),
                    )
                    ps2 = ps2pool.tile([128, J], f32)
                    for k in range(4):
                        dd, s = k // 2, k % 2
                        nc.tensor.matmul(
                            ps2[:],
                            t1[:, 2 * dd + s],
                            hs_sb[:, dd, s, :],
                            start=(k == 0),
                            stop=(k == 3),
                        )
                    nc.vector.tensor_copy(out=out_tile[:, g, :], in_=ps2[:])
                for h in range(2):
                    nc.sync.dma_start(
                        out=yv[cb][:, h * (G // 2) : (h + 1) * (G // 2)],
                        in_=out_tile[:, h * (G // 2) : (h + 1) * (G // 2)],
                    )

    nc.compile()
    return nc


def _get_program():
    if "nc" not in _PROG_CACHE:
        _PROG_CACHE["nc"] = _build_program()
    return _PROG_CACHE["nc"]


def kernel(x, _trace=False, _trace_kwargs=None):
    from concourse.bass_utils import run_bass_kernel_spmd

    x = np.ascontiguousarray(np.asarray(x, dtype=np.float32))
    assert x.shape == (N_ROWS, DIM), x.shape

    nc = _get_program()
    core_ids = list(range(N_CORES))
    in_maps = [
        {"x": x[c * R_PER_CORE : (c + 1) * R_PER_CORE]} for c in core_ids
    ]
    res = run_bass_kernel_spmd(
        nc, in_maps, core_ids, trace=_trace, **(_trace_kwargs or {})
    )
    out = np.concatenate([r["y"] for r in res.results], axis=0)
    if _trace:
        return out, res
    return out
